# revision 11
# baseline (speedup 1.0000x reference)
"""Trainium2 Bass kernel for nn_Decoding_43404939493634 (gnn_message_passing).

Reference computation:
    Z_a = node_embedding[actions_idx]            # [B, 64] gather
    s   = state_embedding @ W_4                  # [B, 1]
    Q   = relu(Z_a * s) @ W_5                    # [B, 1]

Algebraic restructuring: for a row with scalar s,
    relu(z * s) @ W5 = s * (relu(z) @ W5)        if s > 0
                     = s * (min(z, 0) @ W5)      if s <= 0
so with per-node values a0 = relu(node)@W5, a1 = min(node,0)@W5 (computed
on device), the per-batch-row work collapses to
    Q[b] = s[b] * ((s[b] > 0) * d[idx[b]] + a1[idx[b]]),   d = a0 - a1

Two SPMD launches on 8 cores, all streams host-staged in bf16 (halves
HBM read traffic; same numerics as the previous inline-cast design):

  launch A (fused stream):
    - node path (PE): only the REFERENCED node rows (~21.6k/core, 2.8
      MiB/core) staged transposed in a "slot" layout [128, cols] (column
      n of chunk c holds two rows' embeddings on partitions 0-63 /
      64-127); bf16 matmuls with a tiny block-diagonal stationary land
      each chunk's dot products on its own pair of PSUM partitions.
      relu on DVE, PSUM evacuation via ScalarE Copy + DVE/GpSimd subs.
    - state path (DVE only — keeps the Tensor engine off the critical
      path; its clock ramps from ~1.2 to ~2.4 GHz only after ~16 us):
      state staged ROW-major [128 partitions, 392 rows x 64 emb] (a
      plain host reshape), multiplied by a repeated-W4 block and
      sum-reduced per 64-column group on DVE -> s [128, 392].
    Streams interleave on the two HWDGE queues (sync/act) alternating.
  host: t2 = tbl[actions_idx] gather (data movement only).
  launch B (combine, tiny): loads s + t2, computes
      q = s * ((s>0)*d + a1) on DVE in pipelined column halves.

Host work is data movement only (pad/reshape/permute/take/unique/cast);
every arithmetic op runs on device.
"""

import sys

for _p in ("/opt/trn_rl_repo",):
    if _p not in sys.path:
        sys.path.insert(0, _p)

import numpy as np

import concourse.bacc as bacc
import concourse.mybir as mybir
import concourse.tile as tile

F32 = mybir.dt.float32
BF16 = mybir.dt.bfloat16
ALU = mybir.AluOpType
COPY = mybir.ActivationFunctionType.Copy
AXX = mybir.AxisListType.X
P = 128

N_NODES = 200000
BATCH = 400000
EMB = 64
NCORES = 8

BATCH_PC = BATCH // NCORES           # 50000 rows/core

FD = 512                             # node matmul moving free dim
CHUNK_ROWS = 2 * FD                  # node rows ("slots") per matmul

SGC = 392                            # state rows per partition (128*392
S_SLOTS = P * SGC                    #  = 50176 >= 50000)
S_COLS = SGC * EMB                   # 25088 bf16 cols of state stream

DMA_COLS = 4096                      # 1 MiB (bf16) per streaming DMA
HEAD_COLS = 1024


def _nc(num_devices):
    return bacc.Bacc(
        "TRN2", target_bir_lowering=False, debug=False, num_devices=num_devices
    )


def _dma_tiles(total_cols, tail=0):
    """Streaming schedule: small first tiles (short latency to first use),
    then 1 MiB tiles; optionally a small `tail` tile last (short tail)."""
    sched = []
    end = total_cols - tail
    c0 = 0
    for w in (FD, HEAD_COLS):
        if c0 < end:
            cw = min(w, end - c0)
            sched.append((c0, cw))
            c0 += cw
    while c0 < end:
        cw = min(DMA_COLS, end - c0)
        sched.append((c0, cw))
        c0 += cw
    if tail:
        sched.append((end, tail))
    return sched


def build_fused(n_chunks, num_devices=NCORES):
    """Launch A: stream nodes + state (bf16); node path on PE ->
    a1 = node@W5 - relu(node)@W5, d = 2*relu(node)@W5 - node@W5;
    state path on DVE -> s = state@W4."""
    n_cols = n_chunks * FD
    n_groups = -(-n_chunks // 16)
    nc = _nc(num_devices)
    ndT = nc.declare_dram_parameter("ndT", [P, n_cols], BF16, isOutput=False)
    stR = nc.declare_dram_parameter("stR", [P, S_COLS], BF16, isOutput=False)
    w4blk = nc.declare_dram_parameter("w4blk", [P, DMA_COLS], BF16, isOutput=False)
    patw5 = nc.declare_dram_parameter("patw5", [P, FD], BF16, isOutput=False)
    d_out = nc.declare_dram_parameter("d_out", [P, FD], F32, isOutput=True)
    a1_out = nc.declare_dram_parameter("a1_out", [P, FD], F32, isOutput=True)
    s_out = nc.declare_dram_parameter("s_out", [P, SGC], F32, isOutput=True)

    with tile.TileContext(nc) as tc:
        with (
            tc.tile_pool(name="const", bufs=1) as cpool,
            tc.tile_pool(name="nwork", bufs=5) as npool,
            tc.tile_pool(name="swork", bufs=3) as spool,
            tc.tile_pool(name="mwork", bufs=2) as mpool,
            tc.tile_pool(name="psum", bufs=1, space="PSUM") as ppool,
        ):
            p5 = cpool.tile([P, FD], BF16, tag="p5")
            nc.scalar.dma_start(out=p5[:], in_=patw5[:])
            w4t = cpool.tile([P, DMA_COLS // EMB, EMB], BF16, tag="w4t")
            nc.sync.dma_start(out=w4t[:], in_=w4blk[:])

            ps_a0 = [ppool.tile([P, FD], F32, tag=f"ps_a0{g}", name=f"ps_a0{g}")
                     for g in range(n_groups)]
            ps_s5 = [ppool.tile([P, FD], F32, tag=f"ps_s5{g}", name=f"ps_s5{g}")
                     for g in range(n_groups)]
            ps_w = ppool.tile([P, FD], F32, tag="ps_w")

            # PE warmup: dummy matmuls during the head window so the PE
            # clock ramp (1.2 -> 2.4 GHz) starts as early as possible.
            warm = cpool.tile([P, FD], BF16, tag="warm")
            nc.vector.memset(warm[:], 0.0)
            for _ in range(7):
                nc.tensor.matmul(
                    ps_w[0:32, :], warm[:, 0:32], warm[:],
                    start=True, stop=True, skip_group_check=True,
                    tile_position=(0, 0),
                )

            # evacuation staging tiles
            s_sb = cpool.tile([P, SGC], F32, tag="s_sb")
            a0t = cpool.tile([P, FD], F32, tag="a0t")
            a1t = cpool.tile([P, FD], F32, tag="a1t")
            dt_ = cpool.tile([P, FD], F32, tag="dt")

            # unified stream schedule: alternate state/node tiles
            nsched = [("n",) + t for t in _dma_tiles(n_cols)]
            ssched = [("s",) + t for t in _dma_tiles(S_COLS, tail=FD)]
            sched = []
            while ssched or nsched:
                if ssched:
                    sched.append(ssched.pop(0))
                if nsched:
                    sched.append(nsched.pop(0))

            n_left = [min(16, n_chunks - 16 * g) for g in range(n_groups)]

            qi = 0
            for kind, c0, cw in sched:
                eng = nc.sync if (qi % 2 == 0) else nc.scalar
                oeng = nc.scalar if (qi % 2 == 0) else nc.sync
                qi += 1
                if kind == "s":
                    gw = cw // EMB
                    g0 = c0 // EMB
                    tb = spool.tile([P, gw, EMB], BF16, tag="sb")
                    eng.dma_start(out=tb[:], in_=stR[:, c0:c0 + cw])
                    mt = mpool.tile([P, gw, EMB], BF16, tag="mt")
                    nc.vector.tensor_tensor(
                        out=mt[:], in0=tb[:], in1=w4t[:, 0:gw, :], op=ALU.mult
                    )
                    nc.vector.tensor_reduce(
                        out=s_sb[:, g0:g0 + gw], in_=mt[:], axis=AXX, op=ALU.add
                    )
                    oeng.dma_start(
                        out=s_out[:, g0:g0 + gw], in_=s_sb[:, g0:g0 + gw]
                    )
                    continue
                tb = npool.tile([P, cw], BF16, tag="nb")
                eng.dma_start(out=tb[:], in_=ndT[:, c0:c0 + cw])
                rl = npool.tile([P, cw], BF16, tag="rl")
                nc.vector.tensor_scalar_max(out=rl[:], in0=tb[:], scalar1=0.0)
                base = c0 // FD
                done = []
                for k in range(cw // FD):
                    c = base + k
                    g, j = divmod(c, 16)
                    st_flags = dict(
                        start=(j == 0),
                        stop=(j == 15) or (c == n_chunks - 1),
                        skip_group_check=True,
                        tile_position=(0, 32 * g),
                    )
                    nc.tensor.matmul(
                        ps_a0[g][32 * g:32 * g + 32, :],
                        p5[:, 32 * j:32 * j + 32],
                        rl[:, k * FD:(k + 1) * FD],
                        **st_flags,
                    )
                    nc.tensor.matmul(
                        ps_s5[g][32 * g:32 * g + 32, :],
                        p5[:, 32 * j:32 * j + 32],
                        tb[:, k * FD:(k + 1) * FD],
                        **st_flags,
                    )
                    n_left[g] -= 1
                    if n_left[g] == 0:
                        done.append(g)
                # evacuate completed node PSUM groups (PSUM reads on
                # scalar/DVE — gpsimd cannot touch PSUM)
                for g in done:
                    sl = slice(32 * g, 32 * g + 32)
                    nc.scalar.activation(
                        out=a0t[sl, :], in_=ps_a0[g][sl, :], func=COPY
                    )
                    nc.vector.tensor_tensor(
                        out=a1t[sl, :], in0=ps_s5[g][sl, :], in1=a0t[sl, :],
                        op=ALU.subtract,
                    )
                    nc.gpsimd.tensor_tensor(
                        out=dt_[sl, :], in0=a0t[sl, :], in1=a1t[sl, :],
                        op=ALU.subtract,
                    )
                    nc.scalar.dma_start(out=a1_out[sl, :], in_=a1t[sl, :])
                    nc.scalar.dma_start(out=d_out[sl, :], in_=dt_[sl, :])
    nc.compile()
    return nc


def build_combine(num_devices=NCORES):
    """Launch B: q = s * ((s>0)*d + a1), with (d, a1) host-gathered per row."""
    nc = _nc(num_devices)
    sv = nc.declare_dram_parameter("sv", [P, SGC], F32, isOutput=False)
    t2 = nc.declare_dram_parameter("t2", [P, SGC, 2], F32, isOutput=False)
    q = nc.declare_dram_parameter("q", [P, SGC], F32, isOutput=True)

    NH = 2                                   # column halves, pipelined
    HW_ = SGC // NH
    with tile.TileContext(nc) as tc:
        with tc.tile_pool(name="const", bufs=1) as cpool:
            svt = cpool.tile([P, SGC], F32, tag="svt")
            t2t = cpool.tile([P, SGC, 2], F32, tag="t2t")
            posm = cpool.tile([P, SGC], F32, tag="posm")
            sel = cpool.tile([P, SGC], F32, tag="sel")
            qt = cpool.tile([P, SGC], F32, tag="qt")
            for h in range(NH):
                cs = slice(h * HW_, (h + 1) * HW_)
                nc.sync.dma_start(out=t2t[:, cs, :], in_=t2[:, cs, :])
                nc.scalar.dma_start(out=svt[:, cs], in_=sv[:, cs])
            for h in range(NH):
                cs = slice(h * HW_, (h + 1) * HW_)
                nc.vector.scalar_tensor_tensor(
                    out=posm[:, cs], in0=svt[:, cs], scalar=0.0,
                    in1=t2t[:, cs, 0], op0=ALU.is_gt, op1=ALU.mult,
                )
                nc.vector.tensor_tensor(
                    out=sel[:, cs], in0=posm[:, cs], in1=t2t[:, cs, 1],
                    op=ALU.add,
                )
                nc.vector.tensor_tensor(
                    out=qt[:, cs], in0=svt[:, cs], in1=sel[:, cs], op=ALU.mult
                )
                nc.sync.dma_start(out=q[:, cs], in_=qt[:, cs])
    nc.compile()
    return nc


# ---------------------------------------------------------------------------
# host-side staging (data movement only) + execution

_CACHE = {}
LAST_RUNS = []  # BassKernelResults of each launch in the last kernel() call


def _runner(key, build_fn):
    if key not in _CACHE:
        _CACHE[key] = build_fn()
    return _CACHE[key]


def _run_spmd(nc, in_maps):
    from concourse.bass_utils import run_bass_kernel_spmd

    r = run_bass_kernel_spmd(nc, in_maps, core_ids=list(range(NCORES)))
    LAST_RUNS.append(r)
    return r.results


def _slotT(rows, n_slots, nch):
    """[n, 64] -> transposed node slot layout [128, n_slots//2] bf16: column
    n of chunk c holds rows (1024c+2n) on partitions 0-63 and (1024c+2n+1)
    on 64-127."""
    n = rows.shape[0]
    buf = np.zeros((n_slots, EMB), np.float32)
    buf[:n] = rows
    arr = buf.reshape(nch, FD, 2, EMB)           # [c, n, h, e]
    return np.ascontiguousarray(
        arr.transpose(2, 3, 0, 1).reshape(P, nch * FD)
    ).astype(mybir.dt.np(BF16))


def _pidx(n_chunks):
    """Partition index of (chunk c, half h) in the node psum output layout."""
    c = np.arange(n_chunks)[:, None]
    h = np.arange(2)[None, :]
    return (32 * (c // 16) + 2 * (c % 16) + h)   # [n_chunks, 2]


def _unslot(mat, n_chunks):
    """[128, 512] device output -> flat [n_chunks*1024] slot-ordered values."""
    pi = _pidx(n_chunks).reshape(-1)             # [2*n_chunks]
    v = mat[pi, :].reshape(n_chunks, 2, FD)      # [c, h, n]
    return np.ascontiguousarray(v.transpose(0, 2, 1)).reshape(-1)


def _patterns(w):
    """16 block-diagonal stationaries packed as [128, 512] bf16: pattern j in
    cols [32j, 32j+32) with w at (rows 0-63, col 2j), (rows 64-127, col
    2j+1)."""
    pat = np.zeros((P, FD), np.float32)
    for j in range(16):
        pat[:EMB, 32 * j + 2 * j] = w
        pat[EMB:, 32 * j + 2 * j + 1] = w
    return pat


def kernel(actions_idx, node_embedding, state_embedding, W_4, W_5):
    LAST_RUNS.clear()
    actions_idx = np.asarray(actions_idx)
    node_embedding = np.ascontiguousarray(np.asarray(node_embedding, dtype=np.float32))
    state_embedding = np.ascontiguousarray(np.asarray(state_embedding, dtype=np.float32))
    w4 = np.asarray(W_4, dtype=np.float32).reshape(EMB)
    w5 = np.asarray(W_5, dtype=np.float32).reshape(EMB)
    bf16 = mybir.dt.np(BF16)
    patw5 = _patterns(w5).astype(bf16)
    w4blk = np.ascontiguousarray(
        np.broadcast_to(np.tile(w4, DMA_COLS // EMB), (P, DMA_COLS))
    ).astype(bf16)

    # ---- launch A: fused node+state stream (only referenced nodes staged)
    uniq, inv = np.unique(actions_idx, return_inverse=True)
    u_pc = -(-len(uniq) // NCORES)               # referenced nodes per core
    n_chunks = max(1, -(-u_pc // CHUNK_ROWS))    # 22 for the target workload
    n_slots = n_chunks * CHUNK_ROWS
    ncA = _runner(("fused", n_chunks), lambda: build_fused(n_chunks))
    inA = []
    sbuf = np.zeros((S_SLOTS, EMB), np.float32)
    for c in range(NCORES):
        rows = node_embedding[uniq[c * u_pc:(c + 1) * u_pc]]
        sbuf[:BATCH_PC] = state_embedding[c * BATCH_PC:(c + 1) * BATCH_PC]
        inA.append({
            "ndT": _slotT(rows, n_slots, n_chunks),
            "stR": sbuf.reshape(P, S_COLS).astype(bf16),
            "w4blk": w4blk,
            "patw5": patw5,
        })
    resA = _run_spmd(ncA, inA)

    tblu = np.empty((NCORES * u_pc, 2), np.float32)
    for c in range(NCORES):
        sl = slice(c * u_pc, (c + 1) * u_pc)
        tblu[sl, 0] = _unslot(resA[c]["d_out"], n_chunks)[:u_pc]
        tblu[sl, 1] = _unslot(resA[c]["a1_out"], n_chunks)[:u_pc]

    # ---- launch B: combine
    ncB = _runner("combine", build_combine)
    inB = []
    for c in range(NCORES):
        cinv = inv[c * BATCH_PC:(c + 1) * BATCH_PC]
        pairs = np.zeros((S_SLOTS, 2), np.float32)
        pairs[:BATCH_PC] = tblu[cinv]
        inB.append({
            "sv": resA[c]["s_out"],
            "t2": pairs.reshape(P, SGC, 2),
        })
    resB = _run_spmd(ncB, inB)

    out = np.empty(BATCH, np.float32)
    for c in range(NCORES):
        out[c * BATCH_PC:(c + 1) * BATCH_PC] = \
            resB[c]["q"].reshape(S_SLOTS)[:BATCH_PC]
    return out.reshape(BATCH, 1)


# revision 12
# speedup vs baseline: 1.1295x; 1.1295x over previous
"""Trainium2 Bass kernel for nn_Decoding_43404939493634 (gnn_message_passing).

Reference computation:
    Z_a = node_embedding[actions_idx]            # [B, 64] gather
    s   = state_embedding @ W_4                  # [B, 1]
    Q   = relu(Z_a * s) @ W_5                    # [B, 1]

Algebraic restructuring: for a row with scalar s,
    relu(z * s) @ W5 = s * (relu(z) @ W5)        if s > 0
                     = s * (min(z, 0) @ W5)      if s <= 0
so with per-node values a0 = relu(node)@W5, a1 = min(node,0)@W5 (computed
on device), the per-batch-row work collapses to
    Q[b] = s[b] * ((s[b] > 0) * d[idx[b]] + a1[idx[b]]),   d = a0 - a1

Two SPMD launches on 8 cores, all streams host-staged in bf16 (halves
HBM read traffic; same numerics as an inline DMA cast):

  launch A (fused stream).  The PE clock ramps ~1.2 -> ~2.4 GHz only
  after ~16 us of activity, so PE cycles are the scarce resource; the
  launch splits the dot-product work between PE and DVE so neither is
  the critical path over the ~10 MiB bf16 stream:
    - node path (PE): the REFERENCED node rows (~21.6k/core) staged
      transposed in a "slot" layout [128, cols] (column n of chunk c
      holds two rows' embeddings on partitions 0-63 / 64-127); bf16
      matmuls with a block-diagonal stationary land each chunk's dots
      on its own pair of PSUM partitions.  relu on DVE, PSUM
      evacuation via ScalarE Copy + DVE/GpSimd subtracts.
    - state segment 1 (PE): first 20480 rows in the same slot layout
      via a W4 stationary -> s1 in PSUM.
    - state segment 2 (DVE): remaining rows staged ROW-major
      [128 partitions, 232 rows x 64 emb] (a plain host reshape),
      multiplied by a repeated-W4 block, then log2-fold summed:
      L1-L3 folds on DVE, L4-L6 on GpSimd -> s2 [128, 232].
    Streams interleave on the two HWDGE queues (sync/act) alternating.
  host: t2 = tbl[actions_idx] gather (data movement only).
  launch B (combine, tiny): loads s1/s2 + t2 segments, computes
      q = s * ((s>0)*d + a1) on DVE, stores q.

Host work is data movement only (pad/reshape/permute/take/unique/cast);
every arithmetic op runs on device.
"""

import sys

for _p in ("/opt/trn_rl_repo",):
    if _p not in sys.path:
        sys.path.insert(0, _p)

import numpy as np

import concourse.bacc as bacc
import concourse.mybir as mybir
import concourse.tile as tile

F32 = mybir.dt.float32
BF16 = mybir.dt.bfloat16
ALU = mybir.AluOpType
COPY = mybir.ActivationFunctionType.Copy
P = 128

N_NODES = 200000
BATCH = 400000
EMB = 64
NCORES = 8

BATCH_PC = BATCH // NCORES           # 50000 rows/core

FD = 512                             # matmul moving free dim / psum cols
CHUNK_ROWS = 2 * FD                  # slot-layout rows per chunk

S_SLOTS = 50176                      # 50000 padded (= 128*392)
S1_CHUNKS = 20                       # state rows on PE (slot layout)
S1_ROWS = S1_CHUNKS * CHUNK_ROWS     # 20480
S1_COLS = S1_CHUNKS * FD             # 10240
NP1 = 40                             # psum partitions used by 20 chunks
S2_ROWS = S_SLOTS - S1_ROWS          # 29696 rows on DVE (row-major)
SGC2 = S2_ROWS // P                  # 232 rows per partition
S2_COLS = SGC2 * EMB                 # 14848

DMA_COLS = 4096                      # 1 MiB (bf16) per streaming DMA
HEAD_COLS = 1024


def _nc(num_devices):
    return bacc.Bacc(
        "TRN2", target_bir_lowering=False, debug=False, num_devices=num_devices
    )


def _dma_tiles(total_cols, tail=0):
    """Streaming schedule: small first tiles (short latency to first use),
    then 1 MiB tiles; optionally a small `tail` tile last (short tail)."""
    sched = []
    end = total_cols - tail
    c0 = 0
    for w in (FD, HEAD_COLS):
        if c0 < end:
            cw = min(w, end - c0)
            sched.append((c0, cw))
            c0 += cw
    while c0 < end:
        cw = min(DMA_COLS, end - c0)
        sched.append((c0, cw))
        c0 += cw
    if tail:
        sched.append((end, tail))
    return sched


def build_fused(n_chunks, num_devices=NCORES):
    """Launch A: stream nodes + state (bf16); nodes + state-seg1 on PE,
    state-seg2 on DVE/GpSimd fold tree."""
    n_cols = n_chunks * FD
    n_groups = -(-n_chunks // 16)
    nc = _nc(num_devices)
    ndT = nc.declare_dram_parameter("ndT", [P, n_cols], BF16, isOutput=False)
    stS1 = nc.declare_dram_parameter("stS1", [P, S1_COLS], BF16, isOutput=False)
    stS2 = nc.declare_dram_parameter("stS2", [P, S2_COLS], BF16, isOutput=False)
    w4blk = nc.declare_dram_parameter("w4blk", [P, DMA_COLS], BF16, isOutput=False)
    patw4 = nc.declare_dram_parameter("patw4", [P, FD], BF16, isOutput=False)
    patw5 = nc.declare_dram_parameter("patw5", [P, FD], BF16, isOutput=False)
    d_out = nc.declare_dram_parameter("d_out", [P, FD], F32, isOutput=True)
    a1_out = nc.declare_dram_parameter("a1_out", [P, FD], F32, isOutput=True)
    s1_out = nc.declare_dram_parameter("s1_out", [NP1, FD], F32, isOutput=True)
    s2_out = nc.declare_dram_parameter("s2_out", [P, SGC2], F32, isOutput=True)

    s1_left = [min(16, S1_CHUNKS - 16 * g) for g in range(2)]
    n_left = [min(16, n_chunks - 16 * g) for g in range(n_groups)]

    with tile.TileContext(nc) as tc:
        with (
            tc.tile_pool(name="const", bufs=1) as cpool,
            tc.tile_pool(name="nwork", bufs=4) as npool,
            tc.tile_pool(name="swork", bufs=3) as spool,
            tc.tile_pool(name="mwork", bufs=2) as mpool,
            tc.tile_pool(name="psum", bufs=1, space="PSUM") as ppool,
        ):
            p4 = cpool.tile([P, FD], BF16, tag="p4")
            nc.sync.dma_start(out=p4[:], in_=patw4[:])
            p5 = cpool.tile([P, FD], BF16, tag="p5")
            nc.scalar.dma_start(out=p5[:], in_=patw5[:])
            w4t = cpool.tile([P, DMA_COLS // EMB, EMB], BF16, tag="w4t")
            nc.sync.dma_start(out=w4t[:], in_=w4blk[:])

            ps_s = [ppool.tile([P, FD], F32, tag=f"ps_s{g}", name=f"ps_s{g}")
                    for g in range(2)]
            ps_a0 = [ppool.tile([P, FD], F32, tag=f"ps_a0{g}", name=f"ps_a0{g}")
                     for g in range(n_groups)]
            ps_s5 = [ppool.tile([P, FD], F32, tag=f"ps_s5{g}", name=f"ps_s5{g}")
                     for g in range(n_groups)]
            ps_w = ppool.tile([P, FD], F32, tag="ps_w")

            # PE warmup: small dummy matmuls so the PE clock ramp starts
            # as early as possible without wasting many cold cycles.
            warm = cpool.tile([P, FD], BF16, tag="warm")
            nc.vector.memset(warm[:], 0.0)
            for _ in range(5):
                nc.tensor.matmul(
                    ps_w[0:32, 0:256], warm[:, 0:32], warm[:, 0:256],
                    start=True, stop=True, skip_group_check=True,
                    tile_position=(0, 0),
                )

            # evacuation staging tiles
            s1_sb = cpool.tile([P, FD], F32, tag="s1_sb")
            s2_sb = cpool.tile([P, SGC2], F32, tag="s2_sb")
            a0t = cpool.tile([P, FD], F32, tag="a0t")
            a1t = cpool.tile([P, FD], F32, tag="a1t")
            dt_ = cpool.tile([P, FD], F32, tag="dt")

            # unified stream schedule, round-robin s2 / node / s1
            scheds = [
                [("2",) + t for t in _dma_tiles(S2_COLS, tail=FD)],
                [("n",) + t for t in _dma_tiles(n_cols)],
                [("1",) + t for t in _dma_tiles(S1_COLS)],
            ]
            sched = []
            while any(scheds):
                for s in scheds:
                    if s:
                        sched.append(s.pop(0))

            qi = 0
            for kind, c0, cw in sched:
                eng = nc.sync if (qi % 2 == 0) else nc.scalar
                oeng = nc.scalar if (qi % 2 == 0) else nc.sync
                qi += 1

                if kind == "2":
                    gw = cw // EMB
                    g0 = c0 // EMB
                    tb = spool.tile([P, gw, EMB], BF16, tag="sb")
                    eng.dma_start(out=tb[:], in_=stS2[:, c0:c0 + cw])
                    mt = mpool.tile([P, gw, EMB], BF16, tag="mt")
                    nc.vector.tensor_tensor(
                        out=mt[:], in0=tb[:], in1=w4t[:, 0:gw, :], op=ALU.mult
                    )
                    f1 = mpool.tile([P, gw, 32], BF16, tag="f1")
                    nc.vector.tensor_tensor(
                        out=f1[:], in0=mt[:, :, 0:32], in1=mt[:, :, 32:64],
                        op=ALU.add)
                    f2 = mpool.tile([P, gw, 16], BF16, tag="f2")
                    nc.vector.tensor_tensor(
                        out=f2[:], in0=f1[:, :, 0:16], in1=f1[:, :, 16:32],
                        op=ALU.add)
                    f3 = mpool.tile([P, gw, 8], BF16, tag="f3")
                    nc.vector.tensor_tensor(
                        out=f3[:], in0=f2[:, :, 0:8], in1=f2[:, :, 8:16],
                        op=ALU.add)
                    f4 = mpool.tile([P, gw, 4], F32, tag="f4")
                    nc.gpsimd.tensor_tensor(
                        out=f4[:], in0=f3[:, :, 0:4], in1=f3[:, :, 4:8],
                        op=ALU.add)
                    f5 = mpool.tile([P, gw, 2], F32, tag="f5")
                    nc.gpsimd.tensor_tensor(
                        out=f5[:], in0=f4[:, :, 0:2], in1=f4[:, :, 2:4],
                        op=ALU.add)
                    nc.gpsimd.tensor_tensor(
                        out=s2_sb[:, g0:g0 + gw], in0=f5[:, :, 0],
                        in1=f5[:, :, 1], op=ALU.add)
                    oeng.dma_start(
                        out=s2_out[:, g0:g0 + gw], in_=s2_sb[:, g0:g0 + gw]
                    )
                    continue

                if kind == "1":
                    tb = spool.tile([P, cw], BF16, tag="s1b")
                    eng.dma_start(out=tb[:], in_=stS1[:, c0:c0 + cw])
                    base = c0 // FD
                    done = []
                    for k in range(cw // FD):
                        c = base + k
                        g, j = divmod(c, 16)
                        nc.tensor.matmul(
                            ps_s[g][32 * g:32 * g + 32, :],
                            p4[:, 32 * j:32 * j + 32],
                            tb[:, k * FD:(k + 1) * FD],
                            start=(j == 0),
                            stop=(j == 15) or (c == S1_CHUNKS - 1),
                            skip_group_check=True,
                            tile_position=(0, 32 * g),
                        )
                        s1_left[g] -= 1
                        if s1_left[g] == 0:
                            done.append(g)
                    for g in done:
                        sl = slice(32 * g, min(32 * g + 32, NP1))
                        nc.scalar.activation(
                            out=s1_sb[sl, :], in_=ps_s[g][sl, :], func=COPY
                        )
                        oeng.dma_start(out=s1_out[sl, :], in_=s1_sb[sl, :])
                    continue

                tb = npool.tile([P, cw], BF16, tag="nb")
                eng.dma_start(out=tb[:], in_=ndT[:, c0:c0 + cw])
                rl = npool.tile([P, cw], BF16, tag="rl")
                nc.vector.tensor_scalar_max(out=rl[:], in0=tb[:], scalar1=0.0)
                base = c0 // FD
                done = []
                for k in range(cw // FD):
                    c = base + k
                    g, j = divmod(c, 16)
                    st_flags = dict(
                        start=(j == 0),
                        stop=(j == 15) or (c == n_chunks - 1),
                        skip_group_check=True,
                        tile_position=(0, 32 * g),
                    )
                    nc.tensor.matmul(
                        ps_a0[g][32 * g:32 * g + 32, :],
                        p5[:, 32 * j:32 * j + 32],
                        rl[:, k * FD:(k + 1) * FD],
                        **st_flags,
                    )
                    nc.tensor.matmul(
                        ps_s5[g][32 * g:32 * g + 32, :],
                        p5[:, 32 * j:32 * j + 32],
                        tb[:, k * FD:(k + 1) * FD],
                        **st_flags,
                    )
                    n_left[g] -= 1
                    if n_left[g] == 0:
                        done.append(g)
                # evacuate completed node PSUM groups (PSUM reads on
                # scalar/DVE — gpsimd cannot touch PSUM)
                for g in done:
                    sl = slice(32 * g, 32 * g + 32)
                    nc.scalar.activation(
                        out=a0t[sl, :], in_=ps_a0[g][sl, :], func=COPY
                    )
                    nc.vector.tensor_tensor(
                        out=a1t[sl, :], in0=ps_s5[g][sl, :], in1=a0t[sl, :],
                        op=ALU.subtract,
                    )
                    nc.gpsimd.tensor_tensor(
                        out=dt_[sl, :], in0=a0t[sl, :], in1=a1t[sl, :],
                        op=ALU.subtract,
                    )
                    nc.scalar.dma_start(out=a1_out[sl, :], in_=a1t[sl, :])
                    nc.scalar.dma_start(out=d_out[sl, :], in_=dt_[sl, :])
    nc.compile()
    return nc


def build_combine(num_devices=NCORES):
    """Launch B: q = s * ((s>0)*d + a1), with (d, a1) host-gathered per row.
    Segment 1 in node-slot layout ([40, 512]); segment 2 row-major."""
    nc = _nc(num_devices)
    sv1 = nc.declare_dram_parameter("sv1", [NP1, FD], F32, isOutput=False)
    t21 = nc.declare_dram_parameter("t21", [NP1, FD, 2], F32, isOutput=False)
    sv2 = nc.declare_dram_parameter("sv2", [P, SGC2], F32, isOutput=False)
    t22 = nc.declare_dram_parameter("t22", [P, SGC2, 2], F32, isOutput=False)
    q1 = nc.declare_dram_parameter("q1", [NP1, FD], F32, isOutput=True)
    q2 = nc.declare_dram_parameter("q2", [P, SGC2], F32, isOutput=True)

    with tile.TileContext(nc) as tc:
        with tc.tile_pool(name="const", bufs=1) as cpool:
            sv1t = cpool.tile([NP1, FD], F32, tag="sv1t")
            t21t = cpool.tile([NP1, FD, 2], F32, tag="t21t")
            sv2t = cpool.tile([P, SGC2], F32, tag="sv2t")
            t22t = cpool.tile([P, SGC2, 2], F32, tag="t22t")
            nc.sync.dma_start(out=t21t[:], in_=t21[:])
            nc.scalar.dma_start(out=sv1t[:], in_=sv1[:])
            nc.sync.dma_start(out=t22t[:], in_=t22[:])
            nc.scalar.dma_start(out=sv2t[:], in_=sv2[:])

            def combine(svt, t2t, qout, shape, tagp):
                posm = cpool.tile(shape, F32, tag=f"po{tagp}")
                sel = cpool.tile(shape, F32, tag=f"se{tagp}")
                qt = cpool.tile(shape, F32, tag=f"qt{tagp}")
                nc.vector.scalar_tensor_tensor(
                    out=posm[:], in0=svt[:], scalar=0.0, in1=t2t[:, :, 0],
                    op0=ALU.is_gt, op1=ALU.mult,
                )
                nc.vector.tensor_tensor(
                    out=sel[:], in0=posm[:], in1=t2t[:, :, 1], op=ALU.add
                )
                nc.vector.tensor_tensor(
                    out=qt[:], in0=svt[:], in1=sel[:], op=ALU.mult
                )
                nc.sync.dma_start(out=qout[:], in_=qt[:])

            combine(sv1t, t21t, q1, [NP1, FD], "1")
            combine(sv2t, t22t, q2, [P, SGC2], "2")
    nc.compile()
    return nc


# ---------------------------------------------------------------------------
# host-side staging (data movement only) + execution

_CACHE = {}
LAST_RUNS = []  # BassKernelResults of each launch in the last kernel() call


def _runner(key, build_fn):
    if key not in _CACHE:
        _CACHE[key] = build_fn()
    return _CACHE[key]


def _run_spmd(nc, in_maps):
    from concourse.bass_utils import run_bass_kernel_spmd

    r = run_bass_kernel_spmd(nc, in_maps, core_ids=list(range(NCORES)))
    LAST_RUNS.append(r)
    return r.results


def _slotT(rows, n_slots, nch):
    """[n, 64] -> transposed slot layout [128, n_slots//2] bf16: column n of
    chunk c holds rows (1024c+2n) on partitions 0-63 / (1024c+2n+1) on
    64-127."""
    n = rows.shape[0]
    buf = np.zeros((n_slots, EMB), np.float32)
    buf[:n] = rows
    arr = buf.reshape(nch, FD, 2, EMB)           # [c, n, h, e]
    return np.ascontiguousarray(
        arr.transpose(2, 3, 0, 1).reshape(P, nch * FD)
    ).astype(mybir.dt.np(BF16))


def _pidx(n_chunks):
    """Partition index of (chunk c, half h) in the psum/slot output layout."""
    c = np.arange(n_chunks)[:, None]
    h = np.arange(2)[None, :]
    return (32 * (c // 16) + 2 * (c % 16) + h)   # [n_chunks, 2]


def _unslot(mat, n_chunks):
    """[>=pmax, 512] device output -> flat [n_chunks*1024] slot-ordered."""
    pi = _pidx(n_chunks).reshape(-1)             # [2*n_chunks]
    v = mat[pi, :].reshape(n_chunks, 2, FD)      # [c, h, n]
    return np.ascontiguousarray(v.transpose(0, 2, 1)).reshape(-1)


def _slot_pairs(pairs, n_chunks, np_out):
    """[n_slots, 2] per-slot values -> [np_out, 512, 2] device layout."""
    pi = _pidx(n_chunks).reshape(-1)
    arr = pairs.reshape(n_chunks, FD, 2, 2)      # [c, n, h, v]
    out = np.zeros((np_out, FD, 2), np.float32)
    out[pi] = arr.transpose(0, 2, 1, 3).reshape(2 * n_chunks, FD, 2)
    return out


def _patterns(w):
    """16 block-diagonal stationaries packed as [128, 512] bf16: pattern j in
    cols [32j, 32j+32) with w at (rows 0-63, col 2j), (rows 64-127, col
    2j+1)."""
    pat = np.zeros((P, FD), np.float32)
    for j in range(16):
        pat[:EMB, 32 * j + 2 * j] = w
        pat[EMB:, 32 * j + 2 * j + 1] = w
    return pat


def kernel(actions_idx, node_embedding, state_embedding, W_4, W_5):
    LAST_RUNS.clear()
    actions_idx = np.asarray(actions_idx)
    node_embedding = np.ascontiguousarray(np.asarray(node_embedding, dtype=np.float32))
    state_embedding = np.ascontiguousarray(np.asarray(state_embedding, dtype=np.float32))
    w4 = np.asarray(W_4, dtype=np.float32).reshape(EMB)
    w5 = np.asarray(W_5, dtype=np.float32).reshape(EMB)
    bf16 = mybir.dt.np(BF16)
    patw4 = _patterns(w4).astype(bf16)
    patw5 = _patterns(w5).astype(bf16)
    w4blk = np.ascontiguousarray(
        np.broadcast_to(np.tile(w4, DMA_COLS // EMB), (P, DMA_COLS))
    ).astype(bf16)

    # ---- launch A: fused node+state stream (only referenced nodes staged)
    uniq, inv = np.unique(actions_idx, return_inverse=True)
    u_pc = -(-len(uniq) // NCORES)               # referenced nodes per core
    n_chunks = max(1, -(-u_pc // CHUNK_ROWS))    # 22 for the target workload
    n_slots = n_chunks * CHUNK_ROWS
    ncA = _runner(("fused", n_chunks), lambda: build_fused(n_chunks))
    inA = []
    s2buf = np.zeros((S2_ROWS, EMB), np.float32)
    for c in range(NCORES):
        rows = node_embedding[uniq[c * u_pc:(c + 1) * u_pc]]
        st = state_embedding[c * BATCH_PC:(c + 1) * BATCH_PC]
        s2buf[:BATCH_PC - S1_ROWS] = st[S1_ROWS:]
        inA.append({
            "ndT": _slotT(rows, n_slots, n_chunks),
            "stS1": _slotT(st[:S1_ROWS], S1_ROWS, S1_CHUNKS),
            "stS2": s2buf.reshape(P, S2_COLS).astype(bf16),
            "w4blk": w4blk,
            "patw4": patw4,
            "patw5": patw5,
        })
    resA = _run_spmd(ncA, inA)

    tblu = np.empty((NCORES * u_pc, 2), np.float32)
    for c in range(NCORES):
        sl = slice(c * u_pc, (c + 1) * u_pc)
        tblu[sl, 0] = _unslot(resA[c]["d_out"], n_chunks)[:u_pc]
        tblu[sl, 1] = _unslot(resA[c]["a1_out"], n_chunks)[:u_pc]

    # ---- launch B: combine
    ncB = _runner("combine", build_combine)
    inB = []
    for c in range(NCORES):
        cinv = inv[c * BATCH_PC:(c + 1) * BATCH_PC]
        pairs = np.zeros((S_SLOTS, 2), np.float32)
        pairs[:BATCH_PC] = tblu[cinv]
        inB.append({
            "sv1": resA[c]["s1_out"],
            "t21": _slot_pairs(pairs[:S1_ROWS], S1_CHUNKS, NP1),
            "sv2": resA[c]["s2_out"],
            "t22": pairs[S1_ROWS:].reshape(P, SGC2, 2),
        })
    resB = _run_spmd(ncB, inB)

    out = np.empty(BATCH, np.float32)
    for c in range(NCORES):
        o = out[c * BATCH_PC:(c + 1) * BATCH_PC]
        o[:S1_ROWS] = _unslot(resB[c]["q1"], S1_CHUNKS)[:S1_ROWS]
        o[S1_ROWS:] = resB[c]["q2"].reshape(S2_ROWS)[:BATCH_PC - S1_ROWS]
    return out.reshape(BATCH, 1)
